# revision 6
# baseline (speedup 1.0000x reference)
"""Multi-head attention (B=2, N=2048, C=768, H=12, DH=64) on 8 Trainium2 cores.

Sharding: data-parallel on batch (cores 0-3 -> b=0, cores 4-7 -> b=1),
tensor-parallel on heads within each group (3 heads/core: Wq/Wk/Wv column
slices, Wp row slices).  Each core emits its partial projection output
[N, C]; the host sums the 4 partials per batch and adds bp.

Per-core dataflow (feature-major, transpose-free, fp16 operands / fp32 psum):
  - host supplies xT = x[b].T  [C, N] in fp16; weight slices arrive
    pre-chunked [128, KC*W] so each loads with a single DMA
  - qT,kT [64, N] per head = W.T @ xT   (heads 0,1 full-M groups; the two
    64-row leftovers of q and k are merged into one M=128 group); biases
    fold into K=1 ones-row matmuls, psum->SBUF casts run on the (else
    idle) scalar engine
  - v [N, 195] token-major with the softmax-denominator ones column baked
    into a zero-gap Wv layout ([v0|1|v1|1|v2|1]), one psum->SBUF copy per
    token tile
  - ST [kj, qi] = kT.T-slice @ qT (scores, transposed); two K=64 matmuls
    packed on disjoint PE row halves stream CONCURRENTLY per [128,1024]
    psum tile (heads 0+1 paired; head 2 pairs even/odd kj)
  - ET = exp(ST - 4) one ACT op per [128,1024]; the ACT engine paces the
    attention inner loop, so all other work hides in its shadow
  - yT_aug[65, qi] = [v_h | 1].T @ ET accumulated over kj; row 64 = denom
  - normalize: reciprocal_approx_fast of denom row, stride-0 DMA
    broadcast (PE ones-broadcast at the tail, off the DMA latency path),
    fused mul-copy
  - out[qi, C] partial = yT @ Wp rows, with the previous block's
    projection matmuls interleaved into the current block's ACT-paced
    stream so the PE never sits idle at block boundaries
"""

import math

import numpy as np

import concourse.bacc as bacc
import concourse.bass as bass
import concourse.mybir as mybir
import concourse.tile as tile
from concourse import bass_utils

B, N, C, H, DH = 2, 2048, 768, 12, 64
NCORES = 8
CPG = 4                  # cores per batch group
HPC = H // CPG           # heads per core = 3
MYC = HPC * DH           # per-core feature width = 192
VW = HPC * 65            # v row width with ones columns = 195
KC = C // 128            # contraction chunks = 6
NTT = N // 128           # token tiles = 16
QB = 512                 # qi block (psum bank width, fp32)
F32 = mybir.dt.float32
MMDT = mybir.dt.float16  # matmul operand dtype: 1cyc/row, 10-bit mantissa
AF = mybir.ActivationFunctionType
OP = mybir.AluOpType

EXP_SHIFT = -4.0         # exp(s + EXP_SHIFT); cancels between num and denom


def _bcast_parts(ap, nparts):
    """Partition-stride-0 broadcast view of a [1, F] AP (DMA source only)."""
    return bass.AP(tensor=ap.tensor, offset=ap.offset,
                   ap=[[0, nparts]] + [list(d) for d in ap.ap[1:]])


def _emit(nc, tc, pools, aps):
    xT, wqA, wkA, wqkB, wv, wp = (
        aps["xT"], aps["wqA"], aps["wkA"], aps["wqkB"], aps["wv"], aps["wp"])
    bqA, bkA, bqkB, bvr, out = (
        aps["bqA"], aps["bkA"], aps["bqkB"], aps["bvr"], aps["out"])
    persist = pools["persist"]
    et_pool = pools["et"]
    small = pools["small"]
    ostage = pools["ostage"]
    dram_bc = pools["dram_bc"]

    # ---- persistent SBUF tensors ----
    xT_sb = persist.tile([128, KC * N], MMDT, tag="xT_sb")
    wqA_sb = persist.tile([128, KC * 128], MMDT, tag="wqA_sb")
    wkA_sb = persist.tile([128, KC * 128], MMDT, tag="wkA_sb")
    wqkB_sb = persist.tile([128, KC * 128], MMDT, tag="wqkB_sb")
    wv_sb = persist.tile([128, KC * VW], MMDT, tag="wv_sb")
    wpA = persist.tile([128, C], MMDT, tag="wpA")
    wpB = persist.tile([64, C], MMDT, tag="wpB")
    bq_row = persist.tile([1, 128], MMDT, tag="bq_row")
    bk_row = persist.tile([1, 128], MMDT, tag="bk_row")
    bqk_row = persist.tile([1, 128], MMDT, tag="bqk_row")
    bvr_sb = persist.tile([1, VW], MMDT, tag="bvr_sb")
    ones = persist.tile([1, 128], MMDT, tag="ones")
    ones512 = persist.tile([1, QB], MMDT, tag="ones512")
    shift_col = persist.tile([128, 1], F32, tag="shift_col")
    qTA = persist.tile([128, N], MMDT, tag="qTA")
    kTA = persist.tile([128, N], MMDT, tag="kTA")
    # head 2 k/q live duplicated on both partition halves (kj even/odd packing)
    qTB = persist.tile([128, N], MMDT, tag="qTB")
    kTB = persist.tile([128, N], MMDT, tag="kTB")
    v_sb = persist.tile([128, NTT * VW], MMDT, tag="v_sb")
    yTA = persist.tile([128, N], MMDT, tag="yTA")
    yTB = persist.tile([64, N], MMDT, tag="yTB")

    # ---- constants (vector) ----
    ones_f32 = persist.tile([1, QB], F32, tag="ones_f32")
    nc.vector.memset(ones_f32, 1.0)
    nc.vector.tensor_copy(out=ones, in_=ones_f32[:, 0:128])
    nc.vector.tensor_copy(out=ones512, in_=ones_f32)
    nc.vector.memset(shift_col, EXP_SHIFT)

    # ---- input DMAs; xT chunks on sync, weights whole on scalar queue ----
    for kc in range(KC):
        nc.sync.dma_start(out=xT_sb[:, kc * N:(kc + 1) * N],
                          in_=xT[kc * 128:(kc + 1) * 128, :])
    nc.scalar.dma_start(out=wqA_sb, in_=wqA)
    nc.scalar.dma_start(out=wkA_sb, in_=wkA)
    nc.scalar.dma_start(out=wqkB_sb, in_=wqkB)
    nc.scalar.dma_start(out=wv_sb, in_=wv)
    nc.scalar.dma_start(out=wpA, in_=wp[0:128, :])
    nc.scalar.dma_start(out=wpB, in_=wp[128:MYC, :])
    nc.scalar.dma_start(out=bq_row, in_=bqA)
    nc.scalar.dma_start(out=bk_row, in_=bkA)
    nc.scalar.dma_start(out=bqk_row, in_=bqkB)
    nc.scalar.dma_start(out=bvr_sb, in_=bvr)

    # ---- phases 1+2: q/k/v projections (own PSUM pool, released after) ----
    with tc.tile_pool(name="ps_proj", bufs=2, space="PSUM") as ps_proj:
        # PE warmup while the first DMAs land: ramps the pstate up
        warm = ps_proj.tile([128, QB], F32, tag="warm", bufs=1)
        for _ in range(8):
            nc.tensor.matmul(warm, ones[0:1, :], ones512, start=True,
                             stop=True)

        for wsb, brow, dsts in (
            (wqA_sb, bq_row, ((qTA, 0, 128),)),
            (wkA_sb, bk_row, ((kTA, 0, 128),)),
            # merged leftover: psum rows 0:64 = q feats 128:192,
            # rows 64:128 = k feats 128:192
            (wqkB_sb, bqk_row, ((qTB, 0, 64), (kTB, 64, 128))),
        ):
            pss = [ps_proj.tile([128, QB], F32, tag="ps_qk", bufs=5,
                                name=f"ps_qk{_i}")
                   for _i in range(N // QB)]
            for kc in range(KC):  # kc outer: overlap the xT load
                for nt in range(N // QB):
                    nc.tensor.matmul(
                        pss[nt],
                        wsb[:, kc * 128:(kc + 1) * 128],
                        xT_sb[:, kc * N + nt * QB: kc * N + nt * QB + QB],
                        start=(kc == 0), stop=False,
                    )
            for nt in range(N // QB):  # K=1 ones-row matmul adds the bias
                nc.tensor.matmul(pss[nt], brow, ones512,
                                 start=False, stop=True)
            for nt in range(N // QB):  # psum->SBUF cast on the scalar engine
                for dst, r0, r1 in dsts:
                    nc.scalar.activation(
                        out=dst[0:r1 - r0, nt * QB:(nt + 1) * QB],
                        in_=pss[nt][r0:r1, :], func=AF.Copy, bias=0.0)
        # duplicate head-2 k/q onto partitions 64..127 (cross-partition: DMA)
        nc.sync.dma_start(out=qTB[64:128, :], in_=qTB[0:64, :])
        nc.sync.dma_start(out=kTB[64:128, :], in_=kTB[0:64, :])

        # v projection: zero-gap wv layout gives psum = [v0|1|v1|1|v2|1]
        for nt in range(NTT):
            ps = ps_proj.tile([128, VW], F32, tag="ps_v")
            for kc in range(KC):
                nc.tensor.matmul(
                    ps,
                    xT_sb[:, kc * N + nt * 128: kc * N + nt * 128 + 128],
                    wv_sb[:, kc * VW:(kc + 1) * VW],
                    start=(kc == 0), stop=False,
                )
            nc.tensor.matmul(ps, ones[0:1, 0:128], bvr_sb,
                             start=False, stop=True)
            nc.vector.tensor_copy(out=v_sb[:, nt * VW:(nt + 1) * VW], in_=ps)

    # ---- phase 3: attention; unit = (head-pair, qi block of 512) ----
    def vh_ap(kj, h):
        base = (kj * HPC + h) * 65
        return v_sb[:, base:base + 65]

    with tc.tile_pool(name="ps_st", bufs=2, space="PSUM") as ps_st, \
         tc.tile_pool(name="ps_yt", bufs=3, space="PSUM") as ps_yt, \
         tc.tile_pool(name="ps_po", bufs=1, space="PSUM") as ps_po:

        def normalize(yt, ydst, q0, bc_ps=None):
            rec = small.tile([1, QB], F32, tag="rec")
            if True:  # exact DVE reciprocal (approx_fast corrupts on HW?)
                nc.vector.reciprocal(out=rec, in_=yt[64:65, :])
            else:
                nc.vector.reciprocal_approx_fast(out=rec, in_=yt[64:65, :])
            if bc_ps is None:  # DMA round-trip broadcast (hidden in-block)
                dr = dram_bc.tile([1, QB], F32)
                nc.sync.dma_start(out=dr, in_=rec)
                bc = small.tile([64, QB], F32, tag="bc_sb")
                nc.sync.dma_start(out=bc, in_=_bcast_parts(dr, 64))
            else:  # PE ones-broadcast into psum (low-latency tail path)
                rec16 = small.tile([1, QB], MMDT, tag="rec16")
                nc.vector.tensor_copy(out=rec16, in_=rec)
                bc_p = bc_ps[0:64, 0:QB]
                nc.tensor.matmul(bc_p, ones[0:1, 0:64], rec16,
                                 start=True, stop=True)
                # stt allows only one PSUM input; idle scalar engine casts
                bc = small.tile([64, QB], F32, tag="bc_sb")
                nc.scalar.activation(out=bc, in_=bc_p, func=AF.Copy, bias=0.0)
            nc.vector.scalar_tensor_tensor(
                out=ydst[:, q0:q0 + QB], in0=yt[0:64, :], scalar=1.0, in1=bc,
                op0=OP.mult, op1=OP.mult,
            )

        # Projection work for block qq arrives as a list of closures; block
        # qq+1's emission drains them into the ACT-paced matmul stream.
        proj_units = []

        def mk_proj_unit(qt, nb, ob):
            def emit(po_tile=None):
                if po_tile is None:
                    po_tile = ps_po.tile([128, QB], F32, tag="po",
                                         name=f"po{qt}_{nb}")
                po = po_tile[:, 0:384]
                nc.tensor.matmul(po, yTA[:, qt * 128:(qt + 1) * 128],
                                 wpA[:, nb * 384:(nb + 1) * 384],
                                 start=True, stop=False)
                nc.tensor.matmul(po, yTB[0:64, qt * 128:(qt + 1) * 128],
                                 wpB[0:64, nb * 384:(nb + 1) * 384],
                                 start=False, stop=True)
                nc.vector.tensor_copy(out=ob[:, nb * 384:(nb + 1) * 384],
                                      in_=po)
                if nb == 1:
                    nc.sync.dma_start(out=out[qt * 128:(qt + 1) * 128, :],
                                      in_=ob)
            return emit

        def queue_proj(qq):
            for qt in range(qq * 4, qq * 4 + 4):
                ob = ostage.tile([128, C], F32, tag="ob", name=f"ob{qt}")
                for nb in range(2):
                    proj_units.append(mk_proj_unit(qt, nb, ob))

        def drain_proj(k):
            for _ in range(min(k, len(proj_units))):
                proj_units.pop(0)()

        for qq in range(4):
            q0 = qq * QB

            # --- head 2, even/odd kj pairs on the PE array halves ---
            yt2 = ps_yt.tile([65, QB], F32, tag="yt")
            prev = None
            for kp in range(NTT // 2):
                kj0, kj1 = 2 * kp, 2 * kp + 1
                st = ps_st.tile([128, 1024], F32, tag="st")
                nc.tensor.matmul(st[:, 0:QB],
                                 kTB[0:64, kj0 * 128:(kj0 + 1) * 128],
                                 qTB[0:64, q0:q0 + QB], start=True, stop=True)
                nc.tensor.matmul(st[:, QB:1024],
                                 kTB[64:128, kj1 * 128:(kj1 + 1) * 128],
                                 qTB[64:128, q0:q0 + QB], start=True, stop=True)
                et = et_pool.tile([128, 1024], MMDT)
                nc.scalar.activation(et, st, AF.Exp, bias=shift_col[:, :])
                if prev is not None:
                    pet, pkp = prev
                    nc.tensor.matmul(yt2, vh_ap(2 * pkp, 2), pet[:, 0:QB],
                                     start=(pkp == 0), stop=False)
                    nc.tensor.matmul(yt2, vh_ap(2 * pkp + 1, 2),
                                     pet[:, QB:1024], start=False, stop=False)
                prev = (et, kp)
                if kp >= 2:  # yT of block qq-1 is normalized ~2 iters in
                    drain_proj(2)
            pet, pkp = prev
            nc.tensor.matmul(yt2, vh_ap(2 * pkp, 2), pet[:, 0:QB],
                             start=(pkp == 0), stop=False)
            nc.tensor.matmul(yt2, vh_ap(2 * pkp + 1, 2), pet[:, QB:1024],
                             start=False, stop=True)
            normalize(yt2, yTB[0:64, :], q0)

            # --- heads 0+1, row-paired on the PE array ---
            yt0 = ps_yt.tile([65, QB], F32, tag="yt")
            yt1 = ps_yt.tile([65, QB], F32, tag="yt")
            prev = None
            for kj in range(NTT):
                st = ps_st.tile([128, 1024], F32, tag="st")
                nc.tensor.matmul(st[:, 0:QB],
                                 kTA[0:64, kj * 128:(kj + 1) * 128],
                                 qTA[0:64, q0:q0 + QB], start=True, stop=True)
                nc.tensor.matmul(st[:, QB:1024],
                                 kTA[64:128, kj * 128:(kj + 1) * 128],
                                 qTA[64:128, q0:q0 + QB], start=True, stop=True)
                et = et_pool.tile([128, 1024], MMDT)
                nc.scalar.activation(et, st, AF.Exp, bias=shift_col[:, :])
                if prev is not None:
                    pet, pkj = prev
                    nc.tensor.matmul(yt0, vh_ap(pkj, 0), pet[:, 0:QB],
                                     start=(pkj == 0), stop=False)
                    nc.tensor.matmul(yt1, vh_ap(pkj, 1), pet[:, QB:1024],
                                     start=(pkj == 0), stop=False)
                prev = (et, kj)
                drain_proj(1)
            pet, pkj = prev
            nc.tensor.matmul(yt0, vh_ap(pkj, 0), pet[:, 0:QB],
                             start=False, stop=True)
            nc.tensor.matmul(yt1, vh_ap(pkj, 1), pet[:, QB:1024],
                             start=False, stop=True)
            if qq < 3:
                normalize(yt0, yTA[0:64, :], q0)
                normalize(yt1, yTA[64:128, :], q0)
            else:  # tail: PE broadcast skips the DMA round-trip latency
                bc0 = ps_yt.tile([65, QB], F32, tag="yt", name="bc0")
                normalize(yt0, yTA[0:64, :], q0, bc_ps=bc0)
                bc1 = ps_po.tile([128, QB], F32, tag="po", name="bc1")
                normalize(yt1, yTA[64:128, :], q0, bc_ps=bc1)

            queue_proj(qq)

        # last block's projection: rotate psum among po + both st slots so
        # the copies pipeline instead of serializing on one bank
        tail_tiles = [None,
                      ps_st.tile([128, 1024], F32, tag="st", name="tp0"),
                      ps_st.tile([128, 1024], F32, tag="st", name="tp1")]
        i = 0
        while proj_units:
            proj_units.pop(0)(po_tile=tail_tiles[i % 3])
            i += 1


def _build_program():
    nc = bacc.Bacc("TRN2", target_bir_lowering=False, debug=False,
                   num_devices=NCORES)
    aps = {
        "xT": nc.dram_tensor("xT", [C, N], MMDT, kind="ExternalInput").ap(),
        # weights arrive pre-chunked: [128, KC*W] with chunk kc at cols
        # kc*W:(kc+1)*W   (host does the (6,128,W)->(128,6,W) transpose)
        "wqA": nc.dram_tensor("wqA", [128, KC * 128], MMDT,
                              kind="ExternalInput").ap(),
        "wkA": nc.dram_tensor("wkA", [128, KC * 128], MMDT,
                              kind="ExternalInput").ap(),
        "wqkB": nc.dram_tensor("wqkB", [128, KC * 128], MMDT,
                               kind="ExternalInput").ap(),
        "wv": nc.dram_tensor("wv", [128, KC * VW], MMDT,
                             kind="ExternalInput").ap(),
        "wp": nc.dram_tensor("wp", [MYC, C], MMDT, kind="ExternalInput").ap(),
        "bqA": nc.dram_tensor("bqA", [1, 128], MMDT,
                              kind="ExternalInput").ap(),
        "bkA": nc.dram_tensor("bkA", [1, 128], MMDT,
                              kind="ExternalInput").ap(),
        "bqkB": nc.dram_tensor("bqkB", [1, 128], MMDT,
                               kind="ExternalInput").ap(),
        "bvr": nc.dram_tensor("bvr", [1, VW], MMDT, kind="ExternalInput").ap(),
        "out": nc.dram_tensor("out", [N, C], F32, kind="ExternalOutput").ap(),
    }
    with tile.TileContext(nc) as tc:
        import contextlib
        with contextlib.ExitStack() as ctx:
            pools = {
                "persist": ctx.enter_context(tc.tile_pool(name="persist", bufs=1)),
                "et": ctx.enter_context(tc.tile_pool(name="et", bufs=3)),
                "small": ctx.enter_context(tc.tile_pool(name="small", bufs=2)),
                "ostage": ctx.enter_context(tc.tile_pool(name="ostage", bufs=3)),
                "dram_bc": ctx.enter_context(
                    tc.tile_pool(name="dram_bc", bufs=2, space="DRAM")),
            }
            _emit(nc, tc, pools, aps)
    nc.compile()
    return nc


_PROGRAM_CACHE = {}


def _get_program():
    if "nc" not in _PROGRAM_CACHE:
        _PROGRAM_CACHE["nc"] = _build_program()
    return _PROGRAM_CACHE["nc"]


def _chunked(w):
    """[C, W] -> [128, KC*W]: chunk kc lands at columns kc*W:(kc+1)*W."""
    wc = np.ascontiguousarray(w)
    return wc.reshape(KC, 128, w.shape[1]).transpose(1, 0, 2).reshape(
        128, KC * w.shape[1])


def make_in_maps(x, Wq, bq, Wk, bk, Wv, bv, Wp, bp):
    scale = 1.0 / math.sqrt(DH)
    xTb = [np.ascontiguousarray(x[b].T) for b in range(B)]
    wire = mybir.dt.np(MMDT)
    in_maps = []
    for c in range(NCORES):
        b, hg = c // CPG, c % CPG
        cols = slice(hg * MYC, (hg + 1) * MYC)
        wq_c = Wq[:, cols] * np.float32(scale)
        wk_c = Wk[:, cols]
        wv_c = Wv[:, cols]
        # zero-gap wv: [v0 | 1-col | v1 | 1-col | v2 | 1-col]; bias row gets
        # the ones so psum comes out in v_sb layout directly
        wv_aug = np.zeros((C, VW), np.float32)
        bv_aug = np.zeros((1, VW), np.float32)
        for h in range(HPC):
            wv_aug[:, h * 65:h * 65 + 64] = wv_c[:, h * DH:(h + 1) * DH]
            bv_aug[0, h * 65:h * 65 + 64] = bv[cols][h * DH:(h + 1) * DH]
            bv_aug[0, h * 65 + 64] = 1.0
        in_maps.append({
            "xT": xTb[b].astype(wire),
            "wqA": _chunked(wq_c[:, 0:128]).astype(wire),
            "wkA": _chunked(wk_c[:, 0:128]).astype(wire),
            "wqkB": _chunked(np.concatenate([wq_c[:, 128:], wk_c[:, 128:]],
                                            axis=1)).astype(wire),
            "wv": _chunked(wv_aug).astype(wire),
            "wp": np.ascontiguousarray(Wp[cols, :]).astype(wire),
            "bqA": (bq[cols][0:128] * np.float32(scale)).reshape(1, 128)
                   .astype(wire),
            "bkA": bk[cols][0:128].reshape(1, 128).astype(wire),
            "bqkB": np.concatenate([bq[cols][128:] * np.float32(scale),
                                    bk[cols][128:]]).reshape(1, 128)
                    .astype(wire),
            "bvr": bv_aug.astype(wire),
        })
    return in_maps


def assemble(results, bp):
    out = np.empty((B, N, C), np.float32)
    for b in range(B):
        acc = results[b * CPG]["out"].astype(np.float64)
        for c in range(b * CPG + 1, (b + 1) * CPG):
            acc = acc + results[c]["out"]
        out[b] = (acc + bp.astype(np.float64)).astype(np.float32)
    return out


def kernel(x, Wq, bq, Wk, bk, Wv, bv, Wp, bp, **extra_kwargs):
    x = np.asarray(x, np.float32)
    Wq = np.asarray(Wq, np.float32)
    Wk = np.asarray(Wk, np.float32)
    Wv = np.asarray(Wv, np.float32)
    Wp = np.asarray(Wp, np.float32)
    bq = np.asarray(bq, np.float32)
    bk = np.asarray(bk, np.float32)
    bv = np.asarray(bv, np.float32)
    bp = np.asarray(bp, np.float32)

    nc = _get_program()
    in_maps = make_in_maps(x, Wq, bq, Wk, bk, Wv, bv, Wp, bp)
    res = bass_utils.run_bass_kernel_spmd(nc, in_maps,
                                          core_ids=list(range(NCORES)))
    return assemble(res.results, bp)


# revision 7
# speedup vs baseline: 1.2176x; 1.2176x over previous
"""Multi-head attention (B=2, N=2048, C=768, H=12, DH=64) on 8 Trainium2 cores.

Sharding: data-parallel on batch (cores 0-3 -> b=0, cores 4-7 -> b=1),
tensor-parallel on heads within each group (3 heads/core: Wq/Wk/Wv column
slices, Wp row slices).  Each core emits its partial projection output
[N, C]; the host sums the 4 partials per batch and adds bp.

Per-core dataflow (feature-major, transpose-free, fp16 operands / fp32 psum):
  - host supplies xT = x[b].T  [C, N] in fp16; weight slices arrive
    pre-chunked [128, KC*W] so each loads with a single DMA
  - qT,kT [64, N] per head = W.T @ xT   (heads 0,1 full-M groups; the two
    64-row leftovers of q and k are merged into one M=128 group); biases
    fold into K=1 ones-row matmuls, psum->SBUF casts run on the (else
    idle) scalar engine
  - v [N, 195] token-major with the softmax-denominator ones column baked
    into a zero-gap Wv layout ([v0|1|v1|1|v2|1]), one psum->SBUF copy per
    token tile
  - ST [kj, qi] = kT.T-slice @ qT (scores, transposed); two K=64 matmuls
    packed on disjoint PE row halves stream CONCURRENTLY per [128,1024]
    psum tile (heads 0+1 paired; head 2 pairs even/odd kj)
  - ET = exp(ST - 4) one ACT op per [128,1024]; the ACT engine paces the
    attention inner loop, so all other work hides in its shadow
  - yT_aug[65, qi] = [v_h | 1].T @ ET accumulated over kj; row 64 = denom
  - normalize: reciprocal_approx_fast of denom row, stride-0 DMA
    broadcast (PE ones-broadcast at the tail, off the DMA latency path),
    fused mul-copy
  - out[qi, C] partial = yT @ Wp rows, with the previous block's
    projection matmuls interleaved into the current block's ACT-paced
    stream so the PE never sits idle at block boundaries
"""

import math

import numpy as np

import concourse.bacc as bacc
import concourse.bass as bass
import concourse.mybir as mybir
import concourse.tile as tile
from concourse import bass_utils

B, N, C, H, DH = 2, 2048, 768, 12, 64
NCORES = 8
CPG = 4                  # cores per batch group
HPC = H // CPG           # heads per core = 3
MYC = HPC * DH           # per-core feature width = 192
VW = HPC * 65            # v row width with ones columns = 195
KC = C // 128            # contraction chunks = 6
NTT = N // 128           # token tiles = 16
QB = 512                 # qi block (psum bank width, fp32)
F32 = mybir.dt.float32
MMDT = mybir.dt.float16  # matmul operand dtype: 1cyc/row, 10-bit mantissa
AF = mybir.ActivationFunctionType
OP = mybir.AluOpType

EXP_SHIFT = -4.0         # exp(s + EXP_SHIFT); cancels between num and denom


def _bcast_parts(ap, nparts):
    """Partition-stride-0 broadcast view of a [1, F] AP (DMA source only)."""
    return bass.AP(tensor=ap.tensor, offset=ap.offset,
                   ap=[[0, nparts]] + [list(d) for d in ap.ap[1:]])


def _emit(nc, tc, pools, aps):
    xT, wqA, wkA, wqkB, wv, wp = (
        aps["xT"], aps["wqA"], aps["wkA"], aps["wqkB"], aps["wv"], aps["wp"])
    bqA, bkA, bqkB, bvr, out = (
        aps["bqA"], aps["bkA"], aps["bqkB"], aps["bvr"], aps["out"])
    persist = pools["persist"]
    et_pool = pools["et"]
    small = pools["small"]
    ostage = pools["ostage"]
    dram_bc = pools["dram_bc"]

    # ---- persistent SBUF tensors ----
    xT_sb = persist.tile([128, KC * N], MMDT, tag="xT_sb")
    wqA_sb = persist.tile([128, KC * 128], MMDT, tag="wqA_sb")
    wkA_sb = persist.tile([128, KC * 128], MMDT, tag="wkA_sb")
    wqkB_sb = persist.tile([128, KC * 128], MMDT, tag="wqkB_sb")
    wv_sb = persist.tile([128, KC * VW], MMDT, tag="wv_sb")
    wpA = persist.tile([128, C], MMDT, tag="wpA")
    wpB = persist.tile([64, C], MMDT, tag="wpB")
    bq_row = persist.tile([1, 128], MMDT, tag="bq_row")
    bk_row = persist.tile([1, 128], MMDT, tag="bk_row")
    bqk_row = persist.tile([1, 128], MMDT, tag="bqk_row")
    bvr_sb = persist.tile([1, VW], MMDT, tag="bvr_sb")
    ones = persist.tile([1, 128], MMDT, tag="ones")
    ones512 = persist.tile([1, QB], MMDT, tag="ones512")
    shift_col = persist.tile([128, 1], F32, tag="shift_col")
    qTA = persist.tile([128, N], MMDT, tag="qTA")
    kTA = persist.tile([128, N], MMDT, tag="kTA")
    # head 2 k/q live duplicated on both partition halves (kj even/odd packing)
    qTB = persist.tile([128, N], MMDT, tag="qTB")
    kTB = persist.tile([128, N], MMDT, tag="kTB")
    v_sb = persist.tile([128, NTT * VW], MMDT, tag="v_sb")
    yTA = persist.tile([128, N], MMDT, tag="yTA")
    yTB = persist.tile([64, N], MMDT, tag="yTB")

    # ---- constants (vector) ----
    ones_f32 = persist.tile([1, QB], F32, tag="ones_f32")
    nc.vector.memset(ones_f32, 1.0)
    nc.vector.tensor_copy(out=ones, in_=ones_f32[:, 0:128])
    nc.vector.tensor_copy(out=ones512, in_=ones_f32)
    nc.vector.memset(shift_col, EXP_SHIFT)

    # ---- input DMAs; xT chunks on sync, weights whole on scalar queue ----
    for kc in range(KC):
        nc.sync.dma_start(out=xT_sb[:, kc * N:(kc + 1) * N],
                          in_=xT[kc * 128:(kc + 1) * 128, :])
    nc.scalar.dma_start(out=wqA_sb, in_=wqA)
    nc.scalar.dma_start(out=wkA_sb, in_=wkA)
    nc.scalar.dma_start(out=wqkB_sb, in_=wqkB)
    nc.scalar.dma_start(out=wv_sb, in_=wv)
    nc.scalar.dma_start(out=wpA, in_=wp[0:128, :])
    nc.scalar.dma_start(out=wpB, in_=wp[128:MYC, :])
    nc.scalar.dma_start(out=bq_row, in_=bqA)
    nc.scalar.dma_start(out=bk_row, in_=bkA)
    nc.scalar.dma_start(out=bqk_row, in_=bqkB)
    nc.scalar.dma_start(out=bvr_sb, in_=bvr)

    # ---- phases 1+2: q/k/v projections (own PSUM pool, released after) ----
    with tc.tile_pool(name="ps_proj", bufs=2, space="PSUM") as ps_proj:
        # PE warmup while the first DMAs land: ramps the pstate up
        warm = ps_proj.tile([128, QB], F32, tag="warm", bufs=1)
        for _ in range(8):
            nc.tensor.matmul(warm, ones[0:1, :], ones512, start=True,
                             stop=True)

        for wsb, brow, dsts in (
            (wqA_sb, bq_row, ((qTA, 0, 128),)),
            (wkA_sb, bk_row, ((kTA, 0, 128),)),
            # merged leftover: psum rows 0:64 = q feats 128:192,
            # rows 64:128 = k feats 128:192
            (wqkB_sb, bqk_row, ((qTB, 0, 64), (kTB, 64, 128))),
        ):
            pss = [ps_proj.tile([128, QB], F32, tag="ps_qk", bufs=5,
                                name=f"ps_qk{_i}")
                   for _i in range(N // QB)]
            for kc in range(KC):  # kc outer: overlap the xT load
                for nt in range(N // QB):
                    nc.tensor.matmul(
                        pss[nt],
                        wsb[:, kc * 128:(kc + 1) * 128],
                        xT_sb[:, kc * N + nt * QB: kc * N + nt * QB + QB],
                        start=(kc == 0), stop=False,
                    )
            for nt in range(N // QB):  # K=1 ones-row matmul adds the bias
                nc.tensor.matmul(pss[nt], brow, ones512,
                                 start=False, stop=True)
            for nt in range(N // QB):  # psum->SBUF cast on the scalar engine
                for dst, r0, r1 in dsts:
                    nc.scalar.activation(
                        out=dst[0:r1 - r0, nt * QB:(nt + 1) * QB],
                        in_=pss[nt][r0:r1, :], func=AF.Copy, bias=0.0)
        # duplicate head-2 k/q onto partitions 64..127 (cross-partition: DMA)
        nc.sync.dma_start(out=qTB[64:128, :], in_=qTB[0:64, :])
        nc.sync.dma_start(out=kTB[64:128, :], in_=kTB[0:64, :])

        # v projection: zero-gap wv layout gives psum = [v0|1|v1|1|v2|1]
        for nt in range(NTT):
            ps = ps_proj.tile([128, VW], F32, tag="ps_v")
            for kc in range(KC):
                nc.tensor.matmul(
                    ps,
                    xT_sb[:, kc * N + nt * 128: kc * N + nt * 128 + 128],
                    wv_sb[:, kc * VW:(kc + 1) * VW],
                    start=(kc == 0), stop=False,
                )
            nc.tensor.matmul(ps, ones[0:1, 0:128], bvr_sb,
                             start=False, stop=True)
            nc.vector.tensor_copy(out=v_sb[:, nt * VW:(nt + 1) * VW], in_=ps)

    # ---- phase 3: attention; unit = (head-pair, qi block of 512) ----
    def vh_ap(kj, h):
        base = (kj * HPC + h) * 65
        return v_sb[:, base:base + 65]

    with tc.tile_pool(name="ps_st", bufs=2, space="PSUM") as ps_st, \
         tc.tile_pool(name="ps_yt", bufs=3, space="PSUM") as ps_yt, \
         tc.tile_pool(name="ps_po", bufs=1, space="PSUM") as ps_po:

        def normalize(yt, ydst, q0, bc_ps=None):
            # approx_fast's BITWISE_NOT seed misreads PSUM inputs on HW:
            # stage the denominator row to SBUF first
            den = small.tile([1, QB], F32, tag="den")
            nc.vector.tensor_copy(out=den, in_=yt[64:65, :])
            rec = small.tile([1, QB], F32, tag="rec")
            nc.vector.reciprocal_approx_fast(out=rec, in_=den)
            if bc_ps is None:  # DMA round-trip broadcast (hidden in-block)
                dr = dram_bc.tile([1, QB], F32)
                nc.sync.dma_start(out=dr, in_=rec)
                bc = small.tile([64, QB], F32, tag="bc_sb")
                nc.sync.dma_start(out=bc, in_=_bcast_parts(dr, 64))
            else:  # PE ones-broadcast into psum (low-latency tail path)
                rec16 = small.tile([1, QB], MMDT, tag="rec16")
                nc.vector.tensor_copy(out=rec16, in_=rec)
                bc_p = bc_ps[0:64, 0:QB]
                nc.tensor.matmul(bc_p, ones[0:1, 0:64], rec16,
                                 start=True, stop=True)
                # stt allows only one PSUM input; idle scalar engine casts
                bc = small.tile([64, QB], F32, tag="bc_sb")
                nc.scalar.activation(out=bc, in_=bc_p, func=AF.Copy, bias=0.0)
            nc.vector.scalar_tensor_tensor(
                out=ydst[:, q0:q0 + QB], in0=yt[0:64, :], scalar=1.0, in1=bc,
                op0=OP.mult, op1=OP.mult,
            )

        # Projection work for block qq arrives as a list of closures; block
        # qq+1's emission drains them into the ACT-paced matmul stream.
        proj_units = []

        def mk_proj_unit(qt, nb, ob):
            def emit(po_tile=None):
                if po_tile is None:
                    po_tile = ps_po.tile([128, QB], F32, tag="po",
                                         name=f"po{qt}_{nb}")
                po = po_tile[:, 0:384]
                nc.tensor.matmul(po, yTA[:, qt * 128:(qt + 1) * 128],
                                 wpA[:, nb * 384:(nb + 1) * 384],
                                 start=True, stop=False)
                nc.tensor.matmul(po, yTB[0:64, qt * 128:(qt + 1) * 128],
                                 wpB[0:64, nb * 384:(nb + 1) * 384],
                                 start=False, stop=True)
                nc.vector.tensor_copy(out=ob[:, nb * 384:(nb + 1) * 384],
                                      in_=po)
                if nb == 1:
                    nc.sync.dma_start(out=out[qt * 128:(qt + 1) * 128, :],
                                      in_=ob)
            return emit

        def queue_proj(qq):
            for qt in range(qq * 4, qq * 4 + 4):
                ob = ostage.tile([128, C], F32, tag="ob", name=f"ob{qt}")
                for nb in range(2):
                    proj_units.append(mk_proj_unit(qt, nb, ob))

        def drain_proj(k):
            for _ in range(min(k, len(proj_units))):
                proj_units.pop(0)()

        for qq in range(4):
            q0 = qq * QB

            # --- head 2, even/odd kj pairs on the PE array halves ---
            yt2 = ps_yt.tile([65, QB], F32, tag="yt")
            prev = None
            for kp in range(NTT // 2):
                kj0, kj1 = 2 * kp, 2 * kp + 1
                st = ps_st.tile([128, 1024], F32, tag="st")
                nc.tensor.matmul(st[:, 0:QB],
                                 kTB[0:64, kj0 * 128:(kj0 + 1) * 128],
                                 qTB[0:64, q0:q0 + QB], start=True, stop=True)
                nc.tensor.matmul(st[:, QB:1024],
                                 kTB[64:128, kj1 * 128:(kj1 + 1) * 128],
                                 qTB[64:128, q0:q0 + QB], start=True, stop=True)
                et = et_pool.tile([128, 1024], MMDT)
                nc.scalar.activation(et, st, AF.Exp, bias=shift_col[:, :])
                if prev is not None:
                    pet, pkp = prev
                    nc.tensor.matmul(yt2, vh_ap(2 * pkp, 2), pet[:, 0:QB],
                                     start=(pkp == 0), stop=False)
                    nc.tensor.matmul(yt2, vh_ap(2 * pkp + 1, 2),
                                     pet[:, QB:1024], start=False, stop=False)
                prev = (et, kp)
                if kp >= 2:  # yT of block qq-1 is normalized ~2 iters in
                    drain_proj(2)
            pet, pkp = prev
            nc.tensor.matmul(yt2, vh_ap(2 * pkp, 2), pet[:, 0:QB],
                             start=(pkp == 0), stop=False)
            nc.tensor.matmul(yt2, vh_ap(2 * pkp + 1, 2), pet[:, QB:1024],
                             start=False, stop=True)
            normalize(yt2, yTB[0:64, :], q0)

            # --- heads 0+1, row-paired on the PE array ---
            yt0 = ps_yt.tile([65, QB], F32, tag="yt")
            yt1 = ps_yt.tile([65, QB], F32, tag="yt")
            prev = None
            for kj in range(NTT):
                st = ps_st.tile([128, 1024], F32, tag="st")
                nc.tensor.matmul(st[:, 0:QB],
                                 kTA[0:64, kj * 128:(kj + 1) * 128],
                                 qTA[0:64, q0:q0 + QB], start=True, stop=True)
                nc.tensor.matmul(st[:, QB:1024],
                                 kTA[64:128, kj * 128:(kj + 1) * 128],
                                 qTA[64:128, q0:q0 + QB], start=True, stop=True)
                et = et_pool.tile([128, 1024], MMDT)
                nc.scalar.activation(et, st, AF.Exp, bias=shift_col[:, :])
                if prev is not None:
                    pet, pkj = prev
                    nc.tensor.matmul(yt0, vh_ap(pkj, 0), pet[:, 0:QB],
                                     start=(pkj == 0), stop=False)
                    nc.tensor.matmul(yt1, vh_ap(pkj, 1), pet[:, QB:1024],
                                     start=(pkj == 0), stop=False)
                prev = (et, kj)
                drain_proj(1)
            pet, pkj = prev
            nc.tensor.matmul(yt0, vh_ap(pkj, 0), pet[:, 0:QB],
                             start=False, stop=True)
            nc.tensor.matmul(yt1, vh_ap(pkj, 1), pet[:, QB:1024],
                             start=False, stop=True)
            if qq < 3:
                normalize(yt0, yTA[0:64, :], q0)
                normalize(yt1, yTA[64:128, :], q0)
            else:  # tail: PE broadcast skips the DMA round-trip latency
                bc0 = ps_yt.tile([65, QB], F32, tag="yt", name="bc0")
                normalize(yt0, yTA[0:64, :], q0, bc_ps=bc0)
                bc1 = ps_po.tile([128, QB], F32, tag="po", name="bc1")
                normalize(yt1, yTA[64:128, :], q0, bc_ps=bc1)

            queue_proj(qq)

        # last block's projection: rotate psum among po + both st slots so
        # the copies pipeline instead of serializing on one bank
        tail_tiles = [None,
                      ps_st.tile([128, 1024], F32, tag="st", name="tp0"),
                      ps_st.tile([128, 1024], F32, tag="st", name="tp1")]
        i = 0
        while proj_units:
            proj_units.pop(0)(po_tile=tail_tiles[i % 3])
            i += 1


def _build_program():
    nc = bacc.Bacc("TRN2", target_bir_lowering=False, debug=False,
                   num_devices=NCORES)
    aps = {
        "xT": nc.dram_tensor("xT", [C, N], MMDT, kind="ExternalInput").ap(),
        # weights arrive pre-chunked: [128, KC*W] with chunk kc at cols
        # kc*W:(kc+1)*W   (host does the (6,128,W)->(128,6,W) transpose)
        "wqA": nc.dram_tensor("wqA", [128, KC * 128], MMDT,
                              kind="ExternalInput").ap(),
        "wkA": nc.dram_tensor("wkA", [128, KC * 128], MMDT,
                              kind="ExternalInput").ap(),
        "wqkB": nc.dram_tensor("wqkB", [128, KC * 128], MMDT,
                               kind="ExternalInput").ap(),
        "wv": nc.dram_tensor("wv", [128, KC * VW], MMDT,
                             kind="ExternalInput").ap(),
        "wp": nc.dram_tensor("wp", [MYC, C], MMDT, kind="ExternalInput").ap(),
        "bqA": nc.dram_tensor("bqA", [1, 128], MMDT,
                              kind="ExternalInput").ap(),
        "bkA": nc.dram_tensor("bkA", [1, 128], MMDT,
                              kind="ExternalInput").ap(),
        "bqkB": nc.dram_tensor("bqkB", [1, 128], MMDT,
                               kind="ExternalInput").ap(),
        "bvr": nc.dram_tensor("bvr", [1, VW], MMDT, kind="ExternalInput").ap(),
        "out": nc.dram_tensor("out", [N, C], F32, kind="ExternalOutput").ap(),
    }
    with tile.TileContext(nc) as tc:
        import contextlib
        with contextlib.ExitStack() as ctx:
            pools = {
                "persist": ctx.enter_context(tc.tile_pool(name="persist", bufs=1)),
                "et": ctx.enter_context(tc.tile_pool(name="et", bufs=3)),
                "small": ctx.enter_context(tc.tile_pool(name="small", bufs=2)),
                "ostage": ctx.enter_context(tc.tile_pool(name="ostage", bufs=3)),
                "dram_bc": ctx.enter_context(
                    tc.tile_pool(name="dram_bc", bufs=2, space="DRAM")),
            }
            _emit(nc, tc, pools, aps)
    nc.compile()
    return nc


_PROGRAM_CACHE = {}


def _get_program():
    if "nc" not in _PROGRAM_CACHE:
        _PROGRAM_CACHE["nc"] = _build_program()
    return _PROGRAM_CACHE["nc"]


def _chunked(w):
    """[C, W] -> [128, KC*W]: chunk kc lands at columns kc*W:(kc+1)*W."""
    wc = np.ascontiguousarray(w)
    return wc.reshape(KC, 128, w.shape[1]).transpose(1, 0, 2).reshape(
        128, KC * w.shape[1])


def make_in_maps(x, Wq, bq, Wk, bk, Wv, bv, Wp, bp):
    scale = 1.0 / math.sqrt(DH)
    xTb = [np.ascontiguousarray(x[b].T) for b in range(B)]
    wire = mybir.dt.np(MMDT)
    in_maps = []
    for c in range(NCORES):
        b, hg = c // CPG, c % CPG
        cols = slice(hg * MYC, (hg + 1) * MYC)
        wq_c = Wq[:, cols] * np.float32(scale)
        wk_c = Wk[:, cols]
        wv_c = Wv[:, cols]
        # zero-gap wv: [v0 | 1-col | v1 | 1-col | v2 | 1-col]; bias row gets
        # the ones so psum comes out in v_sb layout directly
        wv_aug = np.zeros((C, VW), np.float32)
        bv_aug = np.zeros((1, VW), np.float32)
        for h in range(HPC):
            wv_aug[:, h * 65:h * 65 + 64] = wv_c[:, h * DH:(h + 1) * DH]
            bv_aug[0, h * 65:h * 65 + 64] = bv[cols][h * DH:(h + 1) * DH]
            bv_aug[0, h * 65 + 64] = 1.0
        in_maps.append({
            "xT": xTb[b].astype(wire),
            "wqA": _chunked(wq_c[:, 0:128]).astype(wire),
            "wkA": _chunked(wk_c[:, 0:128]).astype(wire),
            "wqkB": _chunked(np.concatenate([wq_c[:, 128:], wk_c[:, 128:]],
                                            axis=1)).astype(wire),
            "wv": _chunked(wv_aug).astype(wire),
            "wp": np.ascontiguousarray(Wp[cols, :]).astype(wire),
            "bqA": (bq[cols][0:128] * np.float32(scale)).reshape(1, 128)
                   .astype(wire),
            "bkA": bk[cols][0:128].reshape(1, 128).astype(wire),
            "bqkB": np.concatenate([bq[cols][128:] * np.float32(scale),
                                    bk[cols][128:]]).reshape(1, 128)
                    .astype(wire),
            "bvr": bv_aug.astype(wire),
        })
    return in_maps


def assemble(results, bp):
    out = np.empty((B, N, C), np.float32)
    for b in range(B):
        acc = results[b * CPG]["out"].astype(np.float64)
        for c in range(b * CPG + 1, (b + 1) * CPG):
            acc = acc + results[c]["out"]
        out[b] = (acc + bp.astype(np.float64)).astype(np.float32)
    return out


def kernel(x, Wq, bq, Wk, bk, Wv, bv, Wp, bp, **extra_kwargs):
    x = np.asarray(x, np.float32)
    Wq = np.asarray(Wq, np.float32)
    Wk = np.asarray(Wk, np.float32)
    Wv = np.asarray(Wv, np.float32)
    Wp = np.asarray(Wp, np.float32)
    bq = np.asarray(bq, np.float32)
    bk = np.asarray(bk, np.float32)
    bv = np.asarray(bv, np.float32)
    bp = np.asarray(bp, np.float32)

    nc = _get_program()
    in_maps = make_in_maps(x, Wq, bq, Wk, bk, Wv, bv, Wp, bp)
    res = bass_utils.run_bass_kernel_spmd(nc, in_maps,
                                          core_ids=list(range(NCORES)))
    return assemble(res.results, bp)


# revision 9
# speedup vs baseline: 1.2546x; 1.0305x over previous
"""Multi-head attention (B=2, N=2048, C=768, H=12, DH=64) on 8 Trainium2 cores.

Sharding: data-parallel on batch (cores 0-3 -> b=0, cores 4-7 -> b=1),
tensor-parallel on heads within each group (3 heads/core: Wq/Wk/Wv column
slices, Wp row slices).  Each core emits its partial projection output
[N, C]; the host sums the 4 partials per batch and adds bp.

Per-core dataflow (feature-major, transpose-free, fp16 operands / fp32 psum):
  - host supplies xT = x[b].T  [C, N] in fp16; weight slices arrive
    pre-chunked [128, KC*W] so each loads with a single DMA
  - qT,kT [64, N] per head = W.T @ xT   (heads 0,1 full-M groups; the two
    64-row leftovers of q and k are merged into one M=128 group); biases
    fold into K=1 ones-row matmuls, psum->SBUF casts run on the (else
    idle) scalar engine
  - v [N, 195] token-major with the softmax-denominator ones column baked
    into a zero-gap Wv layout ([v0|1|v1|1|v2|1]), one psum->SBUF copy per
    token tile
  - ST [kj, qi] = kT.T-slice @ qT (scores, transposed); two K=64 matmuls
    packed on disjoint PE row halves stream CONCURRENTLY per [128,1024]
    psum tile (heads 0+1 paired; head 2 pairs even/odd kj)
  - ET = exp(ST - 4) one ACT op per [128,1024]; the ACT engine paces the
    attention inner loop, so all other work hides in its shadow
  - yT_aug[65, qi] = [v_h | 1].T @ ET accumulated over kj; row 64 = denom
  - normalize: reciprocal_approx_fast of denom row, stride-0 DMA
    broadcast (PE ones-broadcast at the tail, off the DMA latency path),
    fused mul-copy
  - out[qi, C] partial = yT @ Wp rows, with the previous block's
    projection matmuls interleaved into the current block's ACT-paced
    stream so the PE never sits idle at block boundaries
"""

import math

import numpy as np

import concourse.bacc as bacc
import concourse.bass as bass
import concourse.mybir as mybir
import concourse.tile as tile
from concourse import bass_utils

B, N, C, H, DH = 2, 2048, 768, 12, 64
NCORES = 8
CPG = 4                  # cores per batch group
HPC = H // CPG           # heads per core = 3
MYC = HPC * DH           # per-core feature width = 192
VW = HPC * 65            # v row width with ones columns = 195
KC = C // 128            # contraction chunks = 6
NTT = N // 128           # token tiles = 16
QB = 512                 # qi block (psum bank width, fp32)
F32 = mybir.dt.float32
MMDT = mybir.dt.float16  # matmul operand dtype: 1cyc/row, 10-bit mantissa
AF = mybir.ActivationFunctionType
OP = mybir.AluOpType

EXP_SHIFT = -4.0         # exp(s + EXP_SHIFT); cancels between num and denom


def _bcast_parts(ap, nparts):
    """Partition-stride-0 broadcast view of a [1, F] AP (DMA source only)."""
    return bass.AP(tensor=ap.tensor, offset=ap.offset,
                   ap=[[0, nparts]] + [list(d) for d in ap.ap[1:]])


def _emit(nc, tc, pools, aps):
    xT, wqA, wkA, wqkB, wv, wp = (
        aps["xT"], aps["wqA"], aps["wkA"], aps["wqkB"], aps["wv"], aps["wp"])
    bqA, bkA, bqkB, bvr, out = (
        aps["bqA"], aps["bkA"], aps["bqkB"], aps["bvr"], aps["out"])
    persist = pools["persist"]
    et_pool = pools["et"]
    small = pools["small"]
    ostage = pools["ostage"]
    dram_bc = pools["dram_bc"]

    # ---- persistent SBUF tensors ----
    xT_sb = persist.tile([128, KC * N], MMDT, tag="xT_sb")
    wqA_sb = persist.tile([128, KC * 128], MMDT, tag="wqA_sb")
    wkA_sb = persist.tile([128, KC * 128], MMDT, tag="wkA_sb")
    wqkB_sb = persist.tile([128, KC * 128], MMDT, tag="wqkB_sb")
    wv_sb = persist.tile([128, KC * VW], MMDT, tag="wv_sb")
    wpA = persist.tile([128, C], MMDT, tag="wpA")
    wpB = persist.tile([64, C], MMDT, tag="wpB")
    bq_row = persist.tile([1, 128], MMDT, tag="bq_row")
    bk_row = persist.tile([1, 128], MMDT, tag="bk_row")
    bqk_row = persist.tile([1, 128], MMDT, tag="bqk_row")
    bvr_sb = persist.tile([1, VW], MMDT, tag="bvr_sb")
    ones = persist.tile([1, 128], MMDT, tag="ones")
    ones512 = persist.tile([1, QB], MMDT, tag="ones512")
    shift_col = persist.tile([128, 1], F32, tag="shift_col")
    qTA = persist.tile([128, N], MMDT, tag="qTA")
    kTA = persist.tile([128, N], MMDT, tag="kTA")
    # head 2 k/q live duplicated on both partition halves (kj even/odd packing)
    qTB = persist.tile([128, N], MMDT, tag="qTB")
    kTB = persist.tile([128, N], MMDT, tag="kTB")
    v_sb = persist.tile([128, NTT * VW], MMDT, tag="v_sb")
    yTA = persist.tile([128, N], MMDT, tag="yTA")
    yTB = persist.tile([64, N], MMDT, tag="yTB")

    # ---- constants (vector) ----
    ones_f32 = persist.tile([1, QB], F32, tag="ones_f32")
    nc.vector.memset(ones_f32, 1.0)
    nc.vector.tensor_copy(out=ones, in_=ones_f32[:, 0:128])
    nc.vector.tensor_copy(out=ones512, in_=ones_f32)
    nc.vector.memset(shift_col, EXP_SHIFT)

    # ---- input DMAs; xT chunks on sync, weights whole on scalar queue ----
    for kc in range(KC):
        nc.sync.dma_start(out=xT_sb[:, kc * N:(kc + 1) * N],
                          in_=xT[kc * 128:(kc + 1) * 128, :])
    nc.scalar.dma_start(out=wqA_sb, in_=wqA)
    nc.scalar.dma_start(out=wkA_sb, in_=wkA)
    nc.scalar.dma_start(out=wqkB_sb, in_=wqkB)
    nc.scalar.dma_start(out=wv_sb, in_=wv)
    nc.scalar.dma_start(out=wpA, in_=wp[0:128, :])
    nc.scalar.dma_start(out=wpB, in_=wp[128:MYC, :])
    nc.scalar.dma_start(out=bq_row, in_=bqA)
    nc.scalar.dma_start(out=bk_row, in_=bkA)
    nc.scalar.dma_start(out=bqk_row, in_=bqkB)
    nc.scalar.dma_start(out=bvr_sb, in_=bvr)

    # ---- phases 1+2: q/k/v projections (own PSUM pool, released after) ----
    with tc.tile_pool(name="ps_proj", bufs=2, space="PSUM") as ps_proj:
        # PE warmup while the first DMAs land: ramps the pstate up
        warm = ps_proj.tile([128, QB], F32, tag="warm", bufs=1)
        for _ in range(8):
            nc.tensor.matmul(warm, ones[0:1, :], ones512, start=True,
                             stop=True)

        for wsb, brow, dsts in (
            (wqA_sb, bq_row, ((qTA, 0, 128),)),
            (wkA_sb, bk_row, ((kTA, 0, 128),)),
            # merged leftover: psum rows 0:64 = q feats 128:192,
            # rows 64:128 = k feats 128:192
            (wqkB_sb, bqk_row, ((qTB, 0, 64), (kTB, 64, 128))),
        ):
            pss = [ps_proj.tile([128, QB], F32, tag="ps_qk", bufs=5,
                                name=f"ps_qk{_i}")
                   for _i in range(N // QB)]
            for kc in range(KC):  # kc outer: overlap the xT load
                for nt in range(N // QB):
                    nc.tensor.matmul(
                        pss[nt],
                        wsb[:, kc * 128:(kc + 1) * 128],
                        xT_sb[:, kc * N + nt * QB: kc * N + nt * QB + QB],
                        start=(kc == 0), stop=False,
                    )
            for nt in range(N // QB):  # K=1 ones-row matmul adds the bias
                nc.tensor.matmul(pss[nt], brow, ones512,
                                 start=False, stop=True)
            for nt in range(N // QB):  # psum->SBUF cast on the scalar engine
                for dst, r0, r1 in dsts:
                    nc.scalar.activation(
                        out=dst[0:r1 - r0, nt * QB:(nt + 1) * QB],
                        in_=pss[nt][r0:r1, :], func=AF.Copy, bias=0.0)
        # duplicate head-2 k/q onto partitions 64..127 (cross-partition: DMA)
        nc.sync.dma_start(out=qTB[64:128, :], in_=qTB[0:64, :])
        nc.sync.dma_start(out=kTB[64:128, :], in_=kTB[0:64, :])

        # v projection: zero-gap wv layout gives psum = [v0|1|v1|1|v2|1]
        for nt in range(NTT):
            ps = ps_proj.tile([128, VW], F32, tag="ps_v")
            for kc in range(KC):
                nc.tensor.matmul(
                    ps,
                    xT_sb[:, kc * N + nt * 128: kc * N + nt * 128 + 128],
                    wv_sb[:, kc * VW:(kc + 1) * VW],
                    start=(kc == 0), stop=False,
                )
            nc.tensor.matmul(ps, ones[0:1, 0:128], bvr_sb,
                             start=False, stop=True)
            nc.vector.tensor_copy(out=v_sb[:, nt * VW:(nt + 1) * VW], in_=ps)

    # ---- phase 3: attention; unit = (head-pair, qi block of 512) ----
    def vh_ap(kj, h):
        base = (kj * HPC + h) * 65
        return v_sb[:, base:base + 65]

    with tc.tile_pool(name="ps_st", bufs=2, space="PSUM") as ps_st, \
         tc.tile_pool(name="ps_yt", bufs=3, space="PSUM") as ps_yt, \
         tc.tile_pool(name="ps_po", bufs=1, space="PSUM") as ps_po:

        def normalize(yt, ydst, q0, bc_ps=None, den_eng=None):
            # approx_fast's BITWISE_NOT seed misreads PSUM inputs on HW:
            # stage the denominator row to SBUF first
            den = small.tile([1, QB], F32, tag="den")
            if den_eng == "scalar":  # idle at the tail; runs parallel to DVE
                nc.scalar.activation(out=den, in_=yt[64:65, :], func=AF.Copy,
                                     bias=0.0)
            else:
                nc.vector.tensor_copy(out=den, in_=yt[64:65, :])
            rec = small.tile([1, QB], F32, tag="rec")
            nc.vector.reciprocal_approx_fast(out=rec, in_=den)
            if bc_ps is None:  # DMA round-trip broadcast (hidden in-block)
                dr = dram_bc.tile([1, QB], F32)
                nc.sync.dma_start(out=dr, in_=rec)
                bc = small.tile([64, QB], F32, tag="bc_sb")
                nc.sync.dma_start(out=bc, in_=_bcast_parts(dr, 64))
            else:  # PE ones-broadcast into psum (low-latency tail path)
                rec16 = small.tile([1, QB], MMDT, tag="rec16")
                nc.vector.tensor_copy(out=rec16, in_=rec)
                bc_p = bc_ps[0:64, 0:QB]
                nc.tensor.matmul(bc_p, ones[0:1, 0:64], rec16,
                                 start=True, stop=True)
                # stt allows only one PSUM input; idle scalar engine casts
                bc = small.tile([64, QB], F32, tag="bc_sb")
                nc.scalar.activation(out=bc, in_=bc_p, func=AF.Copy, bias=0.0)
            nc.vector.scalar_tensor_tensor(
                out=ydst[:, q0:q0 + QB], in0=yt[0:64, :], scalar=1.0, in1=bc,
                op0=OP.mult, op1=OP.mult,
            )

        # Projection work for block qq arrives as a list of closures; block
        # qq+1's emission drains them into the ACT-paced matmul stream.
        proj_units = []

        def mk_proj_unit(qt, nb, ob):
            def emit(po_tile=None):
                if po_tile is None:
                    po_tile = ps_po.tile([128, QB], F32, tag="po",
                                         name=f"po{qt}_{nb}")
                po = po_tile[:, 0:384]
                nc.tensor.matmul(po, yTA[:, qt * 128:(qt + 1) * 128],
                                 wpA[:, nb * 384:(nb + 1) * 384],
                                 start=True, stop=False)
                nc.tensor.matmul(po, yTB[0:64, qt * 128:(qt + 1) * 128],
                                 wpB[0:64, nb * 384:(nb + 1) * 384],
                                 start=False, stop=True)
                nc.vector.tensor_copy(out=ob[:, nb * 384:(nb + 1) * 384],
                                      in_=po)
                if nb == 1:
                    nc.sync.dma_start(out=out[qt * 128:(qt + 1) * 128, :],
                                      in_=ob)
            return emit

        def queue_proj(qq):
            for qt in range(qq * 4, qq * 4 + 4):
                ob = ostage.tile([128, C], F32, tag="ob", name=f"ob{qt}")
                for nb in range(2):
                    proj_units.append(mk_proj_unit(qt, nb, ob))

        def drain_proj(k):
            for _ in range(min(k, len(proj_units))):
                proj_units.pop(0)()

        for qq in range(4):
            q0 = qq * QB

            # --- head 2, even/odd kj pairs on the PE array halves ---
            yt2 = ps_yt.tile([65, QB], F32, tag="yt")
            prev = None
            for kp in range(NTT // 2):
                kj0, kj1 = 2 * kp, 2 * kp + 1
                st = ps_st.tile([128, 1024], F32, tag="st")
                nc.tensor.matmul(st[:, 0:QB],
                                 kTB[0:64, kj0 * 128:(kj0 + 1) * 128],
                                 qTB[0:64, q0:q0 + QB], start=True, stop=True)
                nc.tensor.matmul(st[:, QB:1024],
                                 kTB[64:128, kj1 * 128:(kj1 + 1) * 128],
                                 qTB[64:128, q0:q0 + QB], start=True, stop=True)
                et = et_pool.tile([128, 1024], MMDT)
                nc.scalar.activation(et, st, AF.Exp, bias=shift_col[:, :])
                if prev is not None:
                    pet, pkp = prev
                    nc.tensor.matmul(yt2, vh_ap(2 * pkp, 2), pet[:, 0:QB],
                                     start=(pkp == 0), stop=False)
                    nc.tensor.matmul(yt2, vh_ap(2 * pkp + 1, 2),
                                     pet[:, QB:1024], start=False, stop=False)
                prev = (et, kp)
                if kp >= 2 and kp % 2 == 0:  # yT of qq-1 lands ~2 iters in
                    drain_proj(1)
            pet, pkp = prev
            nc.tensor.matmul(yt2, vh_ap(2 * pkp, 2), pet[:, 0:QB],
                             start=(pkp == 0), stop=False)
            nc.tensor.matmul(yt2, vh_ap(2 * pkp + 1, 2), pet[:, QB:1024],
                             start=False, stop=True)
            normalize(yt2, yTB[0:64, :], q0)

            # --- heads 0+1, row-paired on the PE array ---
            yt0 = ps_yt.tile([65, QB], F32, tag="yt")
            yt1 = ps_yt.tile([65, QB], F32, tag="yt")
            prev = None
            for kj in range(NTT):
                st = ps_st.tile([128, 1024], F32, tag="st")
                nc.tensor.matmul(st[:, 0:QB],
                                 kTA[0:64, kj * 128:(kj + 1) * 128],
                                 qTA[0:64, q0:q0 + QB], start=True, stop=True)
                nc.tensor.matmul(st[:, QB:1024],
                                 kTA[64:128, kj * 128:(kj + 1) * 128],
                                 qTA[64:128, q0:q0 + QB], start=True, stop=True)
                et = et_pool.tile([128, 1024], MMDT)
                nc.scalar.activation(et, st, AF.Exp, bias=shift_col[:, :])
                if prev is not None:
                    pet, pkj = prev
                    nc.tensor.matmul(yt0, vh_ap(pkj, 0), pet[:, 0:QB],
                                     start=(pkj == 0), stop=False)
                    nc.tensor.matmul(yt1, vh_ap(pkj, 1), pet[:, QB:1024],
                                     start=(pkj == 0), stop=False)
                prev = (et, kj)
                if kj % 2 == 0:
                    drain_proj(1)
            pet, pkj = prev
            nc.tensor.matmul(yt0, vh_ap(pkj, 0), pet[:, 0:QB],
                             start=False, stop=True)
            nc.tensor.matmul(yt1, vh_ap(pkj, 1), pet[:, QB:1024],
                             start=False, stop=True)
            if qq < 3:
                normalize(yt0, yTA[0:64, :], q0)
                normalize(yt1, yTA[64:128, :], q0)
            else:  # tail: PE broadcast skips the DMA round-trip latency
                bc0 = ps_yt.tile([65, QB], F32, tag="yt", name="bc0")
                normalize(yt0, yTA[0:64, :], q0, bc_ps=bc0,
                          den_eng="scalar")
                bc1 = ps_po.tile([128, QB], F32, tag="po", name="bc1")
                normalize(yt1, yTA[64:128, :], q0, bc_ps=bc1)

            queue_proj(qq)

        # last block's projection: rotate psum among po + both st slots so
        # the copies pipeline instead of serializing on one bank
        tail_tiles = [None,
                      ps_st.tile([128, 1024], F32, tag="st", name="tp0"),
                      ps_st.tile([128, 1024], F32, tag="st", name="tp1")]
        i = 0
        while proj_units:
            proj_units.pop(0)(po_tile=tail_tiles[i % 3])
            i += 1


def _build_program():
    nc = bacc.Bacc("TRN2", target_bir_lowering=False, debug=False,
                   num_devices=NCORES)
    aps = {
        "xT": nc.dram_tensor("xT", [C, N], MMDT, kind="ExternalInput").ap(),
        # weights arrive pre-chunked: [128, KC*W] with chunk kc at cols
        # kc*W:(kc+1)*W   (host does the (6,128,W)->(128,6,W) transpose)
        "wqA": nc.dram_tensor("wqA", [128, KC * 128], MMDT,
                              kind="ExternalInput").ap(),
        "wkA": nc.dram_tensor("wkA", [128, KC * 128], MMDT,
                              kind="ExternalInput").ap(),
        "wqkB": nc.dram_tensor("wqkB", [128, KC * 128], MMDT,
                               kind="ExternalInput").ap(),
        "wv": nc.dram_tensor("wv", [128, KC * VW], MMDT,
                             kind="ExternalInput").ap(),
        "wp": nc.dram_tensor("wp", [MYC, C], MMDT, kind="ExternalInput").ap(),
        "bqA": nc.dram_tensor("bqA", [1, 128], MMDT,
                              kind="ExternalInput").ap(),
        "bkA": nc.dram_tensor("bkA", [1, 128], MMDT,
                              kind="ExternalInput").ap(),
        "bqkB": nc.dram_tensor("bqkB", [1, 128], MMDT,
                               kind="ExternalInput").ap(),
        "bvr": nc.dram_tensor("bvr", [1, VW], MMDT, kind="ExternalInput").ap(),
        "out": nc.dram_tensor("out", [N, C], F32, kind="ExternalOutput").ap(),
    }
    with tile.TileContext(nc) as tc:
        import contextlib
        with contextlib.ExitStack() as ctx:
            pools = {
                "persist": ctx.enter_context(tc.tile_pool(name="persist", bufs=1)),
                "et": ctx.enter_context(tc.tile_pool(name="et", bufs=3)),
                "small": ctx.enter_context(tc.tile_pool(name="small", bufs=2)),
                "ostage": ctx.enter_context(tc.tile_pool(name="ostage", bufs=3)),
                "dram_bc": ctx.enter_context(
                    tc.tile_pool(name="dram_bc", bufs=2, space="DRAM")),
            }
            _emit(nc, tc, pools, aps)
    nc.compile()
    return nc


_PROGRAM_CACHE = {}


def _get_program():
    if "nc" not in _PROGRAM_CACHE:
        _PROGRAM_CACHE["nc"] = _build_program()
    return _PROGRAM_CACHE["nc"]


def _chunked(w):
    """[C, W] -> [128, KC*W]: chunk kc lands at columns kc*W:(kc+1)*W."""
    wc = np.ascontiguousarray(w)
    return wc.reshape(KC, 128, w.shape[1]).transpose(1, 0, 2).reshape(
        128, KC * w.shape[1])


def make_in_maps(x, Wq, bq, Wk, bk, Wv, bv, Wp, bp):
    scale = 1.0 / math.sqrt(DH)
    xTb = [np.ascontiguousarray(x[b].T) for b in range(B)]
    wire = mybir.dt.np(MMDT)
    in_maps = []
    for c in range(NCORES):
        b, hg = c // CPG, c % CPG
        cols = slice(hg * MYC, (hg + 1) * MYC)
        wq_c = Wq[:, cols] * np.float32(scale)
        wk_c = Wk[:, cols]
        wv_c = Wv[:, cols]
        # zero-gap wv: [v0 | 1-col | v1 | 1-col | v2 | 1-col]; bias row gets
        # the ones so psum comes out in v_sb layout directly
        wv_aug = np.zeros((C, VW), np.float32)
        bv_aug = np.zeros((1, VW), np.float32)
        for h in range(HPC):
            wv_aug[:, h * 65:h * 65 + 64] = wv_c[:, h * DH:(h + 1) * DH]
            bv_aug[0, h * 65:h * 65 + 64] = bv[cols][h * DH:(h + 1) * DH]
            bv_aug[0, h * 65 + 64] = 1.0
        in_maps.append({
            "xT": xTb[b].astype(wire),
            "wqA": _chunked(wq_c[:, 0:128]).astype(wire),
            "wkA": _chunked(wk_c[:, 0:128]).astype(wire),
            "wqkB": _chunked(np.concatenate([wq_c[:, 128:], wk_c[:, 128:]],
                                            axis=1)).astype(wire),
            "wv": _chunked(wv_aug).astype(wire),
            "wp": np.ascontiguousarray(Wp[cols, :]).astype(wire),
            "bqA": (bq[cols][0:128] * np.float32(scale)).reshape(1, 128)
                   .astype(wire),
            "bkA": bk[cols][0:128].reshape(1, 128).astype(wire),
            "bqkB": np.concatenate([bq[cols][128:] * np.float32(scale),
                                    bk[cols][128:]]).reshape(1, 128)
                    .astype(wire),
            "bvr": bv_aug.astype(wire),
        })
    return in_maps


def assemble(results, bp):
    out = np.empty((B, N, C), np.float32)
    for b in range(B):
        acc = results[b * CPG]["out"].astype(np.float64)
        for c in range(b * CPG + 1, (b + 1) * CPG):
            acc = acc + results[c]["out"]
        out[b] = (acc + bp.astype(np.float64)).astype(np.float32)
    return out


def kernel(x, Wq, bq, Wk, bk, Wv, bv, Wp, bp, **extra_kwargs):
    x = np.asarray(x, np.float32)
    Wq = np.asarray(Wq, np.float32)
    Wk = np.asarray(Wk, np.float32)
    Wv = np.asarray(Wv, np.float32)
    Wp = np.asarray(Wp, np.float32)
    bq = np.asarray(bq, np.float32)
    bk = np.asarray(bk, np.float32)
    bv = np.asarray(bv, np.float32)
    bp = np.asarray(bp, np.float32)

    nc = _get_program()
    in_maps = make_in_maps(x, Wq, bq, Wk, bk, Wv, bv, Wp, bp)
    res = bass_utils.run_bass_kernel_spmd(nc, in_maps,
                                          core_ids=list(range(NCORES)))
    return assemble(res.results, bp)


# revision 11
# speedup vs baseline: 1.2633x; 1.0069x over previous
"""Multi-head attention (B=2, N=2048, C=768, H=12, DH=64) on 8 Trainium2 cores.

Sharding: data-parallel on batch (cores 0-3 -> b=0, cores 4-7 -> b=1),
tensor-parallel on heads within each group (3 heads/core: Wq/Wk/Wv column
slices, Wp row slices).  Each core emits its partial projection output
[N, C]; the host sums the 4 partials per batch and adds bp.

Per-core dataflow (feature-major, transpose-free, fp16 operands / fp32 psum):
  - host supplies xT = x[b].T  [C, N] in fp16; weight slices arrive
    pre-chunked [128, KC*W] so each loads with a single DMA
  - qT,kT [64, N] per head = W.T @ xT   (heads 0,1 full-M groups; the two
    64-row leftovers of q and k are merged into one M=128 group); biases
    fold into K=1 ones-row matmuls, psum->SBUF casts run on the (else
    idle) scalar engine
  - v [N, 195] token-major with the softmax-denominator ones column baked
    into a zero-gap Wv layout ([v0|1|v1|1|v2|1]), one psum->SBUF copy per
    token tile
  - ST [kj, qi] = kT.T-slice @ qT (scores, transposed); two K=64 matmuls
    packed on disjoint PE row halves stream CONCURRENTLY per [128,1024]
    psum tile (heads 0+1 paired; head 2 pairs even/odd kj)
  - ET = exp(ST - 4) one ACT op per [128,1024]; the ACT engine paces the
    attention inner loop, so all other work hides in its shadow
  - yT_aug[65, qi] = [v_h | 1].T @ ET accumulated over kj; row 64 = denom
  - normalize: reciprocal_approx_fast of denom row, stride-0 DMA
    broadcast (PE ones-broadcast at the tail, off the DMA latency path),
    fused mul-copy
  - out[qi, C] partial = yT @ Wp rows, with the previous block's
    projection matmuls interleaved into the current block's ACT-paced
    stream so the PE never sits idle at block boundaries
"""

import math

import numpy as np

import concourse.bacc as bacc
import concourse.bass as bass
import concourse.mybir as mybir
import concourse.tile as tile
from concourse import bass_utils

B, N, C, H, DH = 2, 2048, 768, 12, 64
NCORES = 8
CPG = 4                  # cores per batch group
HPC = H // CPG           # heads per core = 3
MYC = HPC * DH           # per-core feature width = 192
VW = HPC * 65            # v row width with ones columns = 195
KC = C // 128            # contraction chunks = 6
NTT = N // 128           # token tiles = 16
QB = 512                 # qi block (psum bank width, fp32)
F32 = mybir.dt.float32
MMDT = mybir.dt.float16  # matmul operand dtype: 1cyc/row, 10-bit mantissa
AF = mybir.ActivationFunctionType
OP = mybir.AluOpType

EXP_SHIFT = -4.0         # exp(s + EXP_SHIFT); cancels between num and denom


def _bcast_parts(ap, nparts):
    """Partition-stride-0 broadcast view of a [1, F] AP (DMA source only)."""
    return bass.AP(tensor=ap.tensor, offset=ap.offset,
                   ap=[[0, nparts]] + [list(d) for d in ap.ap[1:]])


def _emit(nc, tc, pools, aps):
    xT, wqA, wkA, wqkB, wv, wp = (
        aps["xT"], aps["wqA"], aps["wkA"], aps["wqkB"], aps["wv"], aps["wp"])
    bqA, bkA, bqkB, bvr, out = (
        aps["bqA"], aps["bkA"], aps["bqkB"], aps["bvr"], aps["out"])
    persist = pools["persist"]
    et_pool = pools["et"]
    small = pools["small"]
    ostage = pools["ostage"]
    dram_bc = pools["dram_bc"]

    # ---- persistent SBUF tensors ----
    xT_sb = persist.tile([128, KC * N], MMDT, tag="xT_sb")
    wqA_sb = persist.tile([128, KC * 128], MMDT, tag="wqA_sb")
    wkA_sb = persist.tile([128, KC * 128], MMDT, tag="wkA_sb")
    wqkB_sb = persist.tile([128, KC * 128], MMDT, tag="wqkB_sb")
    wv_sb = persist.tile([128, KC * VW], MMDT, tag="wv_sb")
    wpA = persist.tile([128, C], MMDT, tag="wpA")
    wpB = persist.tile([64, C], MMDT, tag="wpB")
    bq_row = persist.tile([1, 128], MMDT, tag="bq_row")
    bk_row = persist.tile([1, 128], MMDT, tag="bk_row")
    bqk_row = persist.tile([1, 128], MMDT, tag="bqk_row")
    bvr_sb = persist.tile([1, VW], MMDT, tag="bvr_sb")
    ones = persist.tile([1, 128], MMDT, tag="ones")
    ones512 = persist.tile([1, QB], MMDT, tag="ones512")
    shift_col = persist.tile([128, 1], F32, tag="shift_col")
    qTA = persist.tile([128, N], MMDT, tag="qTA")
    kTA = persist.tile([128, N], MMDT, tag="kTA")
    # head 2 k/q live duplicated on both partition halves (kj even/odd packing)
    qTB = persist.tile([128, N], MMDT, tag="qTB")
    kTB = persist.tile([128, N], MMDT, tag="kTB")
    v_sb = persist.tile([128, NTT * VW], MMDT, tag="v_sb")
    yTA = persist.tile([128, N], MMDT, tag="yTA")
    yTB = persist.tile([64, N], MMDT, tag="yTB")

    # ---- constants (vector) ----
    ones_f32 = persist.tile([1, QB], F32, tag="ones_f32")
    nc.vector.memset(ones_f32, 1.0)
    nc.vector.tensor_copy(out=ones, in_=ones_f32[:, 0:128])
    nc.vector.tensor_copy(out=ones512, in_=ones_f32)
    nc.vector.memset(shift_col, EXP_SHIFT)

    # ---- input DMAs; xT chunks on sync, weights whole on scalar queue ----
    for kc in range(KC):
        nc.sync.dma_start(out=xT_sb[:, kc * N:(kc + 1) * N],
                          in_=xT[kc * 128:(kc + 1) * 128, :])
    nc.scalar.dma_start(out=wqA_sb, in_=wqA)
    nc.scalar.dma_start(out=wkA_sb, in_=wkA)
    nc.scalar.dma_start(out=wqkB_sb, in_=wqkB)
    nc.scalar.dma_start(out=wv_sb, in_=wv)
    nc.scalar.dma_start(out=wpA, in_=wp[0:128, :])
    nc.scalar.dma_start(out=wpB, in_=wp[128:MYC, :])
    nc.scalar.dma_start(out=bq_row, in_=bqA)
    nc.scalar.dma_start(out=bk_row, in_=bkA)
    nc.scalar.dma_start(out=bqk_row, in_=bqkB)
    nc.scalar.dma_start(out=bvr_sb, in_=bvr)

    # ---- phases 1+2: q/k/v projections (own PSUM pool, released after) ----
    with tc.tile_pool(name="ps_proj", bufs=2, space="PSUM") as ps_proj:
        # PE warmup while the first DMAs land: ramps the pstate up
        warm = ps_proj.tile([128, QB], F32, tag="warm", bufs=1)
        for _ in range(8):
            nc.tensor.matmul(warm, ones[0:1, :], ones512, start=True,
                             stop=True)

        for wsb, brow, dsts in (
            (wqA_sb, bq_row, ((qTA, 0, 128),)),
            (wkA_sb, bk_row, ((kTA, 0, 128),)),
            # merged leftover: psum rows 0:64 = q feats 128:192,
            # rows 64:128 = k feats 128:192
            (wqkB_sb, bqk_row, ((qTB, 0, 64), (kTB, 64, 128))),
        ):
            pss = [ps_proj.tile([128, QB], F32, tag="ps_qk", bufs=5,
                                name=f"ps_qk{_i}")
                   for _i in range(N // QB)]
            for kc in range(KC):  # kc outer: overlap the xT load
                for nt in range(N // QB):
                    nc.tensor.matmul(
                        pss[nt],
                        wsb[:, kc * 128:(kc + 1) * 128],
                        xT_sb[:, kc * N + nt * QB: kc * N + nt * QB + QB],
                        start=(kc == 0), stop=False,
                    )
            for nt in range(N // QB):  # K=1 ones-row matmul adds the bias
                nc.tensor.matmul(pss[nt], brow, ones512,
                                 start=False, stop=True)
            for nt in range(N // QB):  # psum->SBUF cast on the scalar engine
                for dst, r0, r1 in dsts:
                    nc.scalar.activation(
                        out=dst[0:r1 - r0, nt * QB:(nt + 1) * QB],
                        in_=pss[nt][r0:r1, :], func=AF.Copy, bias=0.0)
        # duplicate head-2 k/q onto partitions 64..127 (cross-partition: DMA)
        nc.sync.dma_start(out=qTB[64:128, :], in_=qTB[0:64, :])
        nc.sync.dma_start(out=kTB[64:128, :], in_=kTB[0:64, :])

    # ---- phase 3: attention; unit = (head-pair, qi block of 512) ----
    def vh_ap(kj, h):
        base = (kj * HPC + h) * 65
        return v_sb[:, base:base + 65]

    with tc.tile_pool(name="ps_st", bufs=2, space="PSUM") as ps_st, \
         tc.tile_pool(name="ps_yt", bufs=3, space="PSUM") as ps_yt, \
         tc.tile_pool(name="ps_po", bufs=1, space="PSUM") as ps_po:

        def normalize(yt, ydst, q0, bc_ps=None, den_eng=None):
            # approx_fast's BITWISE_NOT seed misreads PSUM inputs on HW:
            # stage the denominator row to SBUF first
            den = small.tile([1, QB], F32, tag="den")
            if den_eng == "scalar":  # idle at the tail; runs parallel to DVE
                nc.scalar.activation(out=den, in_=yt[64:65, :], func=AF.Copy,
                                     bias=0.0)
            else:
                nc.vector.tensor_copy(out=den, in_=yt[64:65, :])
            rec = small.tile([1, QB], F32, tag="rec")
            nc.vector.reciprocal_approx_fast(out=rec, in_=den)
            if bc_ps is None:  # DMA round-trip broadcast (hidden in-block)
                dr = dram_bc.tile([1, QB], F32)
                nc.sync.dma_start(out=dr, in_=rec)
                bc = small.tile([64, QB], F32, tag="bc_sb")
                nc.sync.dma_start(out=bc, in_=_bcast_parts(dr, 64))
            else:  # PE ones-broadcast into psum (low-latency tail path)
                rec16 = small.tile([1, QB], MMDT, tag="rec16")
                nc.vector.tensor_copy(out=rec16, in_=rec)
                bc_p = bc_ps[0:64, 0:QB]
                nc.tensor.matmul(bc_p, ones[0:1, 0:64], rec16,
                                 start=True, stop=True)
                # stt allows only one PSUM input; idle scalar engine casts
                bc = small.tile([64, QB], F32, tag="bc_sb")
                nc.scalar.activation(out=bc, in_=bc_p, func=AF.Copy, bias=0.0)
            nc.vector.scalar_tensor_tensor(
                out=ydst[:, q0:q0 + QB], in0=yt[0:64, :], scalar=1.0, in1=bc,
                op0=OP.mult, op1=OP.mult,
            )

        # v production unit: emitted inside block 0's h01 pass so the PE's
        # ACT-slack absorbs it; psum borrowed from the (then idle) po slot
        def emit_v(nt):
            ps = ps_po.tile([128, QB], F32, tag="po", name=f"psv{nt}")
            psv = ps[:, 0:VW]
            for kc in range(KC):
                nc.tensor.matmul(
                    psv,
                    xT_sb[:, kc * N + nt * 128: kc * N + nt * 128 + 128],
                    wv_sb[:, kc * VW:(kc + 1) * VW],
                    start=(kc == 0), stop=False,
                )
            nc.tensor.matmul(psv, ones[0:1, 0:128], bvr_sb,
                             start=False, stop=True)
            nc.vector.tensor_copy(out=v_sb[:, nt * VW:(nt + 1) * VW], in_=psv)

        # Projection work for block qq arrives as HALF-units (one matmul
        # each) so a single iteration's ACT slack absorbs each piece;
        # block qq+1's emission drains them into the ACT-paced stream.
        proj_units = []
        po_map = {}

        def drain_proj(k=1, tile=None):
            for _ in range(min(k, len(proj_units))):
                kind, qt, nb, ob = proj_units.pop(0)
                if kind == "A":
                    po_t = tile
                    if po_t is None:
                        po_t = ps_po.tile([128, QB], F32, tag="po",
                                          name=f"po{qt}_{nb}")
                    po_map[(qt, nb)] = po_t
                    nc.tensor.matmul(po_t[:, 0:384],
                                     yTA[:, qt * 128:(qt + 1) * 128],
                                     wpA[:, nb * 384:(nb + 1) * 384],
                                     start=True, stop=False)
                else:
                    po_t = po_map.pop((qt, nb))
                    nc.tensor.matmul(po_t[:, 0:384],
                                     yTB[0:64, qt * 128:(qt + 1) * 128],
                                     wpB[0:64, nb * 384:(nb + 1) * 384],
                                     start=False, stop=True)
                    nc.vector.tensor_copy(out=ob[:, nb * 384:(nb + 1) * 384],
                                          in_=po_t[:, 0:384])
                    if nb == 1:
                        nc.sync.dma_start(out=out[qt * 128:(qt + 1) * 128, :],
                                          in_=ob)

        def queue_proj(qq):
            for qt in range(qq * 4, qq * 4 + 4):
                ob = ostage.tile([128, C], F32, tag="ob", name=f"ob{qt}")
                for nb in range(2):
                    proj_units.append(("A", qt, nb, ob))
                    proj_units.append(("B", qt, nb, ob))

        def h2_pass(qq):
            q0 = qq * QB
            yt2 = ps_yt.tile([65, QB], F32, tag="yt")
            prev = None
            for kp in range(NTT // 2):
                kj0, kj1 = 2 * kp, 2 * kp + 1
                st = ps_st.tile([128, 1024], F32, tag="st")
                nc.tensor.matmul(st[:, 0:QB],
                                 kTB[0:64, kj0 * 128:(kj0 + 1) * 128],
                                 qTB[0:64, q0:q0 + QB], start=True, stop=True)
                nc.tensor.matmul(st[:, QB:1024],
                                 kTB[64:128, kj1 * 128:(kj1 + 1) * 128],
                                 qTB[64:128, q0:q0 + QB], start=True, stop=True)
                et = et_pool.tile([128, 1024], MMDT)
                nc.scalar.activation(et, st, AF.Exp, bias=shift_col[:, :])
                if prev is not None:
                    pet, pkp = prev
                    nc.tensor.matmul(yt2, vh_ap(2 * pkp, 2), pet[:, 0:QB],
                                     start=(pkp == 0), stop=False)
                    nc.tensor.matmul(yt2, vh_ap(2 * pkp + 1, 2),
                                     pet[:, QB:1024], start=False, stop=False)
                prev = (et, kp)
                if kp >= 3:  # yT of qq-1 is normalized ~3 iters in
                    drain_proj(1)
            pet, pkp = prev
            nc.tensor.matmul(yt2, vh_ap(2 * pkp, 2), pet[:, 0:QB],
                             start=(pkp == 0), stop=False)
            nc.tensor.matmul(yt2, vh_ap(2 * pkp + 1, 2), pet[:, QB:1024],
                             start=False, stop=True)
            normalize(yt2, yTB[0:64, :], q0)

        def h01_pass(qq, with_v=False, tail=False):
            q0 = qq * QB
            yt0 = ps_yt.tile([65, QB], F32, tag="yt")
            yt1 = ps_yt.tile([65, QB], F32, tag="yt")
            prev = None
            for kj in range(NTT):
                if with_v:
                    emit_v(kj)
                st = ps_st.tile([128, 1024], F32, tag="st")
                nc.tensor.matmul(st[:, 0:QB],
                                 kTA[0:64, kj * 128:(kj + 1) * 128],
                                 qTA[0:64, q0:q0 + QB], start=True, stop=True)
                nc.tensor.matmul(st[:, QB:1024],
                                 kTA[64:128, kj * 128:(kj + 1) * 128],
                                 qTA[64:128, q0:q0 + QB], start=True, stop=True)
                et = et_pool.tile([128, 1024], MMDT)
                nc.scalar.activation(et, st, AF.Exp, bias=shift_col[:, :])
                if prev is not None:
                    pet, pkj = prev
                    nc.tensor.matmul(yt0, vh_ap(pkj, 0), pet[:, 0:QB],
                                     start=(pkj == 0), stop=False)
                    nc.tensor.matmul(yt1, vh_ap(pkj, 1), pet[:, QB:1024],
                                     start=(pkj == 0), stop=False)
                prev = (et, kj)
                drain_proj(1)
            pet, pkj = prev
            nc.tensor.matmul(yt0, vh_ap(pkj, 0), pet[:, 0:QB],
                             start=False, stop=True)
            nc.tensor.matmul(yt1, vh_ap(pkj, 1), pet[:, QB:1024],
                             start=False, stop=True)
            if not tail:
                normalize(yt0, yTA[0:64, :], q0)
                normalize(yt1, yTA[64:128, :], q0)
            else:  # tail: PE broadcast skips the DMA round-trip latency
                bc0 = ps_yt.tile([65, QB], F32, tag="yt", name="bc0")
                normalize(yt0, yTA[0:64, :], q0, bc_ps=bc0, den_eng="scalar")
                bc1 = ps_po.tile([128, QB], F32, tag="po", name="bc1")
                normalize(yt1, yTA[64:128, :], q0, bc_ps=bc1)

        # block 0: h01 first, with v production riding in its ACT slack
        h01_pass(0, with_v=True)
        h2_pass(0)
        queue_proj(0)
        for qq in range(1, 4):
            h2_pass(qq)
            h01_pass(qq, tail=(qq == 3))
            queue_proj(qq)

        # last block's projection: rotate psum among both st slots + po so
        # the copies pipeline instead of serializing on one bank
        tail_tiles = [ps_st.tile([128, 1024], F32, tag="st", name="tp0"),
                      ps_st.tile([128, 1024], F32, tag="st", name="tp1"),
                      None]
        ti = 0
        while proj_units:
            tile = None
            if proj_units[0][0] == "A":
                tile = tail_tiles[ti % 3]
                ti += 1
            drain_proj(1, tile=tile)


def _build_program():
    nc = bacc.Bacc("TRN2", target_bir_lowering=False, debug=False,
                   num_devices=NCORES)
    aps = {
        "xT": nc.dram_tensor("xT", [C, N], MMDT, kind="ExternalInput").ap(),
        # weights arrive pre-chunked: [128, KC*W] with chunk kc at cols
        # kc*W:(kc+1)*W   (host does the (6,128,W)->(128,6,W) transpose)
        "wqA": nc.dram_tensor("wqA", [128, KC * 128], MMDT,
                              kind="ExternalInput").ap(),
        "wkA": nc.dram_tensor("wkA", [128, KC * 128], MMDT,
                              kind="ExternalInput").ap(),
        "wqkB": nc.dram_tensor("wqkB", [128, KC * 128], MMDT,
                               kind="ExternalInput").ap(),
        "wv": nc.dram_tensor("wv", [128, KC * VW], MMDT,
                             kind="ExternalInput").ap(),
        "wp": nc.dram_tensor("wp", [MYC, C], MMDT, kind="ExternalInput").ap(),
        "bqA": nc.dram_tensor("bqA", [1, 128], MMDT,
                              kind="ExternalInput").ap(),
        "bkA": nc.dram_tensor("bkA", [1, 128], MMDT,
                              kind="ExternalInput").ap(),
        "bqkB": nc.dram_tensor("bqkB", [1, 128], MMDT,
                               kind="ExternalInput").ap(),
        "bvr": nc.dram_tensor("bvr", [1, VW], MMDT, kind="ExternalInput").ap(),
        "out": nc.dram_tensor("out", [N, C], F32, kind="ExternalOutput").ap(),
    }
    with tile.TileContext(nc) as tc:
        import contextlib
        with contextlib.ExitStack() as ctx:
            pools = {
                "persist": ctx.enter_context(tc.tile_pool(name="persist", bufs=1)),
                "et": ctx.enter_context(tc.tile_pool(name="et", bufs=3)),
                "small": ctx.enter_context(tc.tile_pool(name="small", bufs=2)),
                "ostage": ctx.enter_context(tc.tile_pool(name="ostage", bufs=3)),
                "dram_bc": ctx.enter_context(
                    tc.tile_pool(name="dram_bc", bufs=2, space="DRAM")),
            }
            _emit(nc, tc, pools, aps)
    nc.compile()
    return nc


_PROGRAM_CACHE = {}


def _get_program():
    if "nc" not in _PROGRAM_CACHE:
        _PROGRAM_CACHE["nc"] = _build_program()
    return _PROGRAM_CACHE["nc"]


def _chunked(w):
    """[C, W] -> [128, KC*W]: chunk kc lands at columns kc*W:(kc+1)*W."""
    wc = np.ascontiguousarray(w)
    return wc.reshape(KC, 128, w.shape[1]).transpose(1, 0, 2).reshape(
        128, KC * w.shape[1])


def make_in_maps(x, Wq, bq, Wk, bk, Wv, bv, Wp, bp):
    scale = 1.0 / math.sqrt(DH)
    xTb = [np.ascontiguousarray(x[b].T) for b in range(B)]
    wire = mybir.dt.np(MMDT)
    in_maps = []
    for c in range(NCORES):
        b, hg = c // CPG, c % CPG
        cols = slice(hg * MYC, (hg + 1) * MYC)
        wq_c = Wq[:, cols] * np.float32(scale)
        wk_c = Wk[:, cols]
        wv_c = Wv[:, cols]
        # zero-gap wv: [v0 | 1-col | v1 | 1-col | v2 | 1-col]; bias row gets
        # the ones so psum comes out in v_sb layout directly
        wv_aug = np.zeros((C, VW), np.float32)
        bv_aug = np.zeros((1, VW), np.float32)
        for h in range(HPC):
            wv_aug[:, h * 65:h * 65 + 64] = wv_c[:, h * DH:(h + 1) * DH]
            bv_aug[0, h * 65:h * 65 + 64] = bv[cols][h * DH:(h + 1) * DH]
            bv_aug[0, h * 65 + 64] = 1.0
        in_maps.append({
            "xT": xTb[b].astype(wire),
            "wqA": _chunked(wq_c[:, 0:128]).astype(wire),
            "wkA": _chunked(wk_c[:, 0:128]).astype(wire),
            "wqkB": _chunked(np.concatenate([wq_c[:, 128:], wk_c[:, 128:]],
                                            axis=1)).astype(wire),
            "wv": _chunked(wv_aug).astype(wire),
            "wp": np.ascontiguousarray(Wp[cols, :]).astype(wire),
            "bqA": (bq[cols][0:128] * np.float32(scale)).reshape(1, 128)
                   .astype(wire),
            "bkA": bk[cols][0:128].reshape(1, 128).astype(wire),
            "bqkB": np.concatenate([bq[cols][128:] * np.float32(scale),
                                    bk[cols][128:]]).reshape(1, 128)
                    .astype(wire),
            "bvr": bv_aug.astype(wire),
        })
    return in_maps


def assemble(results, bp):
    out = np.empty((B, N, C), np.float32)
    for b in range(B):
        acc = results[b * CPG]["out"].astype(np.float64)
        for c in range(b * CPG + 1, (b + 1) * CPG):
            acc = acc + results[c]["out"]
        out[b] = (acc + bp.astype(np.float64)).astype(np.float32)
    return out


def kernel(x, Wq, bq, Wk, bk, Wv, bv, Wp, bp, **extra_kwargs):
    x = np.asarray(x, np.float32)
    Wq = np.asarray(Wq, np.float32)
    Wk = np.asarray(Wk, np.float32)
    Wv = np.asarray(Wv, np.float32)
    Wp = np.asarray(Wp, np.float32)
    bq = np.asarray(bq, np.float32)
    bk = np.asarray(bk, np.float32)
    bv = np.asarray(bv, np.float32)
    bp = np.asarray(bp, np.float32)

    nc = _get_program()
    in_maps = make_in_maps(x, Wq, bq, Wk, bk, Wv, bv, Wp, bp)
    res = bass_utils.run_bass_kernel_spmd(nc, in_maps,
                                          core_ids=list(range(NCORES)))
    return assemble(res.results, bp)
